# revision 41
# baseline (speedup 1.0000x reference)
"""Bradley-Terry loss kernel for Trainium2 — Chebyshev/PE design, v11.

loss = sum_{i!=j} W[i,j] * softplus(b_j - b_i)
     = sum_{m,l} A[m,l] * z[m,l] - ln2 * trace(W),
  z[m,l] = sum_ij W_ij T_m(x_i) T_l(x_j),  x = (b - c)/h in [-1,1]

W is quantized to fp8e4m3 on the HOST (identical rounding to an
on-device DVE cast, ~3e-4 relative error against a 2e-2 budget), so the
device streams 8 MiB per core instead of 32 MiB — a 4x cut in the HBM
read that paces this memory-bound kernel.

Per core TensorE computes Y[m, j] = sum_{i in shard} W[i, j] T_m(x_i)
with the fp8 Chebyshev basis stationary (DoubleRow, K=256).  VectorE
dots each 512-col psum slab against B[m, j] = sum_l A[m, l] T_l(x_j)
(fp8, [16, N]) with accum_out, yielding a [16, 16] fp32 output that the
host reduces.  deg-15 Chebyshev suffices: truncation is ~1e-10 while
fp8 quantization dominates at ~3e-4.

Layout/ordering (all measured on this part):
- One sync-queue stream in need-order: bmat, then W chunk-major.  A
  second queue steals DMA engines mid-stream and skews completions.
- W is packed [p, ch, tp, ks, c] per column chunk; wide chunks land as
  2-tile-pair DMAs with 8 KiB descriptors (~25 GB/s/engine); the final
  512-col chunk lands per tile-pair so only 1 matmul + 1 dot trail the
  last W byte.
- Every DMA writes a whole tile: partial-tile writes occasionally
  miswired scheduler deps (nan from garbage fp8); a nan guard
  rebuilds+reruns as backstop.
- The whole fp8 shard is SBUF-resident (64 KiB/partition), so the W
  stream never stalls on compute; psum is 8 per-slab [16, 512] banks,
  so each dot waits only on its own slab's closing matmul.
"""

import numpy as np
import ml_dtypes

import concourse.bacc as bacc
import concourse.bass as bass
import concourse.mybir as mybir
from concourse import tile
from concourse.bass_utils import run_bass_kernel_spmd

N = 8192
NCORES = 8
R = N // NCORES            # 1024 rows per core
P = 128                    # SBUF partitions
TPAIR = 4                  # DoubleRow tile-pairs (K=256 each)
SLAB = 512                 # PSUM bank free size (fp32)
WIDTHS = (2048, 2048, 2048, 1024, 512, 512)
COL0 = tuple(sum(WIDTHS[:i]) for i in range(len(WIDTHS)))
NCH = len(WIDTHS)
NSLABS = tuple(w // SLAB for w in WIDTHS)
SLAB0 = tuple(sum(NSLABS[:i]) for i in range(NCH))
NACC = sum(NSLABS)         # 16 accumulator columns
DEG = 15
M1 = DEG + 1               # 16 chebyshev coefficients (truncation ~1e-10;
                           # fp8 quantization dominates at ~3e-4)
_LN2 = float(np.log(2.0))

_cached_nc = None


def _cheb_vals(x, deg):
    out = np.empty((len(x), deg + 1), dtype=np.float64)
    out[:, 0] = 1.0
    if deg >= 1:
        out[:, 1] = x
    for k in range(2, deg + 1):
        out[:, k] = 2 * x * out[:, k - 1] - out[:, k - 2]
    return out


def _cheb2d_coeffs(f, deg):
    n = deg + 1
    theta = (np.arange(n) + 0.5) * np.pi / n
    pts = np.cos(theta)
    F = f(pts[:, None], pts[None, :])
    Tm = np.cos(np.outer(np.arange(n), theta))
    A = (2.0 / n) * Tm @ F @ ((2.0 / n) * Tm).T
    A[0, :] /= 2
    A[:, 0] /= 2
    return A


def _build(tp_only=False):
    nc = bacc.Bacc(
        "TRN2",
        target_bir_lowering=False,
        debug=False,
        enable_asserts=False,
        num_devices=NCORES,
    )
    f32 = mybir.dt.float32
    f8 = mybir.dt.float8e4
    # per-chunk [tp, ks, c] groups concatenated along the free axis
    w8 = nc.dram_tensor("w8", [P, TPAIR * 2 * N], f8, kind="ExternalInput")
    # [p, tp, ks, m] = T_m(x_{tp*256 + ks*128 + p}) in fp8
    crows = nc.dram_tensor("crows", [P, TPAIR * 2 * M1], f8, kind="ExternalInput")
    bmat = nc.dram_tensor("bmat", [M1, N], f8, kind="ExternalInput")
    acc = nc.dram_tensor("acc", [M1, NACC], f32, kind="ExternalOutput")

    with tile.TileContext(nc) as tc:
        with (
            tc.tile_pool(name="consts", bufs=1) as consts,
            tc.tile_pool(name="psum", bufs=8, space="PSUM") as pspool,
        ):
            crows_sb = consts.tile([P, TPAIR * 2 * M1], f8)
            nc.scalar.dma_start(crows_sb[:], crows.ap())
            bmat_sb = consts.tile([M1, N], f8)
            nc.sync.dma_start(bmat_sb[:], bmat.ap())
            acc_sb = consts.tile([M1, NACC], f32)
            scrs = [consts.tile([M1, SLAB], f32, name=f"scr_{i}") for i in range(2)]
            crows_v = crows_sb.rearrange("p (tp ks m) -> p tp ks m", tp=TPAIR, ks=2)

            wtiles = []
            for ci, w in enumerate(WIDTHS):
                view = (
                    w8.ap()[:, COL0[ci] * TPAIR * 2 : (COL0[ci] + w) * TPAIR * 2]
                    .rearrange("p (tp ks c) -> p tp ks c", tp=TPAIR, ks=2)
                )
                if ci < NCH - 1 and not tp_only:
                    row = []
                    for h in range(2):
                        t = consts.tile([P, 2, 2, w], f8, name=f"wt_{ci}_{h}")
                        nc.sync.dma_start(t[:], view[:, 2 * h : 2 * h + 2])
                        row.append(t)
                    wtiles.append(("h", row))
                else:
                    # final chunk lands per tile-pair: only 1 matmul + 1
                    # dot trail the last W byte
                    row = []
                    for tp in range(TPAIR):
                        t = consts.tile([P, 2, w], f8, name=f"wt_{ci}_{tp}")
                        nc.sync.dma_start(t[:], view[:, tp])
                        row.append(t)
                    wtiles.append(("tp", row))

            def rhs(ci, tp, s):
                kind, row = wtiles[ci]
                if kind == "h":
                    return row[tp // 2][:, tp % 2, :, s * SLAB : (s + 1) * SLAB]
                return row[tp][:, :, s * SLAB : (s + 1) * SLAB]

            ndots = 0

            def emit_dot(pt, ci, s):
                nonlocal ndots
                idx = SLAB0[ci] + s
                c0 = COL0[ci] + s * SLAB
                nc.vector.scalar_tensor_tensor(
                    out=scrs[ndots % 2][:],
                    in0=pt[:],
                    scalar=0.0,
                    in1=bmat_sb[:, c0 : c0 + SLAB],
                    op0=mybir.AluOpType.bypass,
                    op1=mybir.AluOpType.mult,
                    accum_out=acc_sb[:, idx : idx + 1],
                )
                ndots += 1

            for ci in range(NCH):
                pss = [
                    pspool.tile([M1, SLAB], f32, tag="ps", name=f"ps_{ci}_{s}")
                    for s in range(NSLABS[ci])
                ]
                for tp in range(TPAIR):
                    lhsT = crows_v[:, tp, :, :]
                    for s in range(NSLABS[ci]):
                        nc.tensor.matmul(
                            pss[s][:],
                            lhsT,
                            rhs(ci, tp, s),
                            start=(tp == 0),
                            stop=(tp == TPAIR - 1),
                            perf_mode=mybir.MatmulPerfMode.DoubleRow,
                        )
                for s in range(NSLABS[ci]):
                    emit_dot(pss[s], ci, s)
            nc.scalar.dma_start(acc.ap(), acc_sb[:])

    nc.compile()
    return nc


def _get_nc(tp_only=False):
    global _cached_nc
    if _cached_nc is None:
        _cached_nc = _build(tp_only=tp_only)
    return _cached_nc


def _host_in_maps(win_matrix, betas):
    b64 = betas.astype(np.float64)
    lo, hi = float(b64.min()), float(b64.max())
    c = 0.5 * (lo + hi)
    h = max(0.5 * (hi - lo) * 1.000001, 1e-12)
    x = (b64 - c) / h
    A = _cheb2d_coeffs(lambda X, Y: np.logaddexp(0.0, h * (Y - X)), DEG)
    C = _cheb_vals(x, DEG)                       # [N, 64] f64
    C8 = C.astype(ml_dtypes.float8_e4m3fn)

    # B[m, j] = sum_l A[m, l] T_l(x_j)
    B = A @ C.T                                  # [64, N] f64
    bmat_np = np.ascontiguousarray(B.astype(ml_dtypes.float8_e4m3fn))

    W8 = win_matrix.astype(ml_dtypes.float8_e4m3fn)

    in_maps = []
    for cc in range(NCORES):
        rows = slice(cc * R, (cc + 1) * R)
        # [p, tp, ks, m] packing of the fp8 basis for DoubleRow K=256
        crows_np = np.ascontiguousarray(
            C8[rows].reshape(TPAIR, 2, P, M1).transpose(2, 0, 1, 3).reshape(P, -1)
        )
        # per chunk: [p, tp, ks, c], concatenated along the free axis
        wc = W8[rows].reshape(TPAIR, 2, P, N).transpose(2, 0, 1, 3)  # [P,tp,ks,N]
        w8_np = np.ascontiguousarray(
            np.concatenate(
                [
                    wc[:, :, :, COL0[ci] : COL0[ci] + WIDTHS[ci]].reshape(P, -1)
                    for ci in range(NCH)
                ],
                axis=1,
            )
        )
        in_maps.append({"w8": w8_np, "crows": crows_np, "bmat": bmat_np})
    return in_maps


def kernel(win_matrix, betas, _trace=False):
    win_matrix = np.asarray(win_matrix, dtype=np.float32)
    betas = np.asarray(betas, dtype=np.float32)
    nc = _get_nc()
    in_maps = _host_in_maps(win_matrix, betas)
    trace_const = _LN2 * float(np.trace(win_matrix.astype(np.float64)))
    # A rare scheduler miscompile (process-hash dependent) can drop a
    # DMA->matmul wait, yielding nan from garbage fp8; rebuild+rerun.
    global _cached_nc
    for _attempt in range(3):
        res = run_bass_kernel_spmd(
            nc, in_maps, core_ids=list(range(NCORES)), trace=_trace
        )
        total = 0.0
        for cc in range(NCORES):
            total += float(res.results[cc]["acc"].astype(np.float64).sum())
        total -= trace_const
        if np.isfinite(total):
            break
        # rebuild a structurally different (equivalent) program so a
        # deterministic in-process miscompile can't repeat
        _cached_nc = None
        nc = _get_nc(tp_only=True)
    if _trace:
        kernel.last_results = res
    return np.array(total, dtype=np.float32)
